# revision 15
# baseline (speedup 1.0000x reference)
"""AdaMemAttention Trainium2 kernel (8 NeuronCores, SPMD).

Sharding: core c -> (batch b = c//2, head-group hg = c%2, heads hg*6..hg*6+6).
Each core: qkv for its 6 heads; exact top-k memory selection (bisection
threshold + scan compaction + dma_scatter_add); attention over
[selected 511 | current 1568] keys; pairwise AllToAll exchanging head
features across token halves; output projection for its token half.
Host reassembles [B, N, C] from per-core [784, 768] outputs.
"""
import sys
sys.path.insert(0, "/opt/trn_rl_repo")
import numpy as np

B, N, C, H, D = 4, 1568, 768, 12, 64
NB, NP = 2048, 1568
KB, KP = 153, 358          # int(512*0.3), int(512*0.7)
HL = 6                     # heads per core
SEL = KB + KP              # 511
NH = N // 2                # tokens per core after exchange (784)
NITER = 30                 # bisection iterations
TRASH = 8064               # int16-safe trash row in sel scratch

_cache = {}


def _build():
    import concourse.bass as bass
    import concourse.bacc as bacc
    import concourse.mybir as mybir
    import concourse.tile as tile

    dt = mybir.dt
    Alu = mybir.AluOpType
    Act = mybir.ActivationFunctionType
    f32, f32r, i16, i32 = dt.float32, dt.float32r, dt.int16, dt.int32

    nc = bacc.Bacc("TRN2", target_bir_lowering=False, debug=False, num_devices=8)

    # ---------------- I/O ----------------
    x_d = nc.dram_tensor("x", [N, C], f32, kind="ExternalInput")
    wqkvT_d = nc.dram_tensor("wqkvT", [C, 1152], f32, kind="ExternalInput")
    wprojT_d = nc.dram_tensor("wprojT", [384, C], f32, kind="ExternalInput")
    bproj_d = nc.dram_tensor("bproj", [1, C], f32, kind="ExternalInput")
    bank_k_d = nc.dram_tensor("bank_k", [HL, NB, D], f32, kind="ExternalInput")
    bank_v_d = nc.dram_tensor("bank_v", [HL, NB, D], f32, kind="ExternalInput")
    prev_k_d = nc.dram_tensor("prev_k", [HL, NP, D], f32, kind="ExternalInput")
    prev_v_d = nc.dram_tensor("prev_v", [HL, NP, D], f32, kind="ExternalInput")
    B48b_d = nc.dram_tensor("B48b", [48, 12], f32, kind="ExternalInput")
    B48p_d = nc.dram_tensor("B48p", [48, 12], f32, kind="ExternalInput")
    B12b_d = nc.dram_tensor("B12b", [12, 48], f32, kind="ExternalInput")
    B12p_d = nc.dram_tensor("B12p", [12, 48], f32, kind="ExternalInput")
    kvec_d = nc.dram_tensor("kvec", [12, 1], f32, kind="ExternalInput")
    BmapB_d = nc.dram_tensor("BmapB", [12, 128], f32, kind="ExternalInput")
    BmapP_d = nc.dram_tensor("BmapP", [12, 128], f32, kind="ExternalInput")
    roffB_d = nc.dram_tensor("roffB", [66, 1], f32, kind="ExternalInput")
    roffP_d = nc.dram_tensor("roffP", [66, 1], f32, kind="ExternalInput")
    ident_d = nc.dram_tensor("ident", [128, 128], f32, kind="ExternalInput")

    out_d = nc.dram_tensor("out", [NH, C], f32, kind="ExternalOutput")
    import os
    DBG = os.environ.get("KDBG", "0") == "1"
    if DBG:
        dbg_q1 = nc.dram_tensor("dbg_q1", [128, 3], f32, kind="ExternalOutput")
        dbg_lo = nc.dram_tensor("dbg_lo", [12, 1], f32, kind="ExternalOutput")
        dbg_sctB = nc.dram_tensor("dbg_sctB", [66, NB], f32, kind="ExternalOutput")
        dbg_qT = nc.dram_tensor("dbg_qT", [128, 3, 512], f32, kind="ExternalOutput")
        dbg_aT = nc.dram_tensor("dbg_aT", [128, 3, 512], f32, kind="ExternalOutput")
        dbg_sel = nc.dram_tensor("dbg_sel", [6144, 128], f32, kind="ExternalOutput")
        dbg_yloc = nc.dram_tensor("dbg_yloc", [N, C], f32, kind="ExternalOutput")
        dbg_yhalf = nc.dram_tensor("dbg_yhalf", [NH, C], f32, kind="ExternalOutput")

    QT = [512, 512, 512, 32]
    MC = [128, 128, 128, 127] + [128] * 12 + [32]   # 17 m-chunks (sel | current)
    NCH = 13

    with tile.TileContext(nc) as tc, \
         tc.tile_pool(name="cst", bufs=1) as cst, \
         tc.tile_pool(name="dram", bufs=1, space="DRAM") as dram, \
         tc.tile_pool(name="bigB", bufs=1) as bigB:

        # ---------- constants ----------
        ident = cst.tile([128, 128], f32); nc.sync.dma_start(ident[:], ident_d[:])
        B48b = cst.tile([48, 12], f32); nc.sync.dma_start(B48b[:], B48b_d[:])
        B48p = cst.tile([48, 12], f32); nc.sync.dma_start(B48p[:], B48p_d[:])
        B12b = cst.tile([12, 48], f32); nc.sync.dma_start(B12b[:], B12b_d[:])
        B12p = cst.tile([12, 48], f32); nc.sync.dma_start(B12p[:], B12p_d[:])
        kvec = cst.tile([12, 1], f32); nc.sync.dma_start(kvec[:], kvec_d[:])
        BmapB = cst.tile([12, 128], f32); nc.sync.dma_start(BmapB[:], BmapB_d[:])
        BmapP = cst.tile([12, 128], f32); nc.sync.dma_start(BmapP[:], BmapP_d[:])
        roffB = cst.tile([66, 1], f32); nc.sync.dma_start(roffB[:], roffB_d[:])
        roffP = cst.tile([66, 1], f32); nc.sync.dma_start(roffP[:], roffP_d[:])
        ones1f = cst.tile([1, 128], f32)
        nc.vector.memset(ones1f[:], 1.0)
        ones1 = cst.tile([1, 128], f32r)
        nc.vector.tensor_copy(ones1[:], ones1f[:])

        # ---------- long-lived attention operands ----------
        qT = bigB.tile([128, 3, N], f32r)
        kTc = bigB.tile([128, 3, N], f32r)
        kTs = bigB.tile([128, 3, 512], f32r)
        v_all = bigB.tile([128, 17 * HL, 65], f32r)

        with tc.tile_pool(name="bigC", bufs=1) as bigC:
            sctB = bigC.tile([66, NB], f32)
            sctP = bigC.tile([66, NP], f32)
            cbb = bigC.tile([48, 256], f32)
            cbp = bigC.tile([48, 196], f32)
            q1 = bigC.tile([128, 3], f32)
            q1blk = bigC.tile([128, 6], f32)
            lo = bigC.tile([12, 1], f32)
            hi = bigC.tile([12, 1], f32)
            mid = bigC.tile([12, 1], f32)
            tpb = bigC.tile([128, 1], f32)
            tpp = bigC.tile([128, 1], f32)

            with tc.tile_pool(name="bigA", bufs=1) as bigA:
                xTr = bigA.tile([128, 6, N], f32r)
                wqr = bigA.tile([128, 6, 1152], f32r)

                # ===== phase A: x/w load, transposes, q1 =====
                with tc.tile_pool(name="pA", bufs=1) as pA, \
                     tc.tile_pool(name="psA", bufs=2, space="PSUM") as psA:
                    wq_q = pA.tile([128, 6, 384], f32)
                    for cc in range(6):
                        wqc = pA.tile([128, 1152], f32, tag="wqc", name="wqc",
                                      bufs=2)
                        nc.sync.dma_start(
                            wqc[:], wqkvT_d[128 * cc:128 * (cc + 1), :])
                        nc.vector.tensor_copy(wqr[:, cc, :], wqc[:])
                        nc.vector.tensor_copy(wq_q[:, cc, :], wqc[:, 0:384])
                    x0 = pA.tile([128, 6], f32)
                    nc.sync.dma_start(
                        x0[:],
                        x_d[0:1, :].rearrange("one (cc p) -> p (one cc)", p=128))
                    for fq in range(3):
                        q1ps = psA.tile([128, 1], f32, space="PSUM",
                                        tag="q1ps", name="q1ps")
                        for cc in range(6):
                            nc.tensor.matmul(
                                q1ps[:], wq_q[:, cc, 128 * fq:128 * (fq + 1)],
                                x0[:, cc:cc + 1],
                                start=(cc == 0), stop=(cc == 5))
                        nc.vector.tensor_copy(q1[:, fq:fq + 1], q1ps[:])
                    nc.vector.memset(q1blk[:], 0.0)
                    for h in range(HL):
                        hb = 64 * (h % 2)
                        nc.vector.tensor_copy(q1blk[hb:hb + 64, h:h + 1],
                                              q1[hb:hb + 64, h // 2:h // 2 + 1])
                    for c in range(NCH):
                        rows = 128 if c < 12 else 32
                        xc = pA.tile([128, C], f32, tag="xc", name="xc", bufs=3)
                        nc.sync.dma_start(xc[0:rows, :],
                                          x_d[128 * c:128 * c + rows, :])
                        for cc in range(6):
                            tpx = psA.tile([128, 128], f32, space="PSUM",
                                           tag="tpx", name="tpx", bufs=2)
                            nc.tensor.transpose(
                                tpx[0:128, 0:rows],
                                xc[0:rows, 128 * cc:128 * (cc + 1)],
                                ident[0:rows, 0:rows])
                            nc.any.tensor_copy(
                                xTr[:, cc, 128 * c:128 * c + rows],
                                tpx[0:128, 0:rows])

                # ===== phase B: memory-bank scoring =====
                nc.vector.memset(sctB[:], -1.0e30)
                nc.vector.memset(sctP[:], -1.0e30)
                with tc.tile_pool(name="pB", bufs=2) as pB, \
                     tc.tile_pool(name="psB", bufs=2, space="PSUM") as psB:
                    for (src_d, n_src, sct) in ((bank_k_d, NB, sctB),
                                                (prev_k_d, NP, sctP)):
                        for j in range(3):
                            for c5 in range((n_src + 511) // 512):
                                w = min(512, n_src - 512 * c5)
                                kT5 = pB.tile([128, 512], f32,
                                              tag="kT5", name="kT5")
                                for hh in range(2):
                                    h = 2 * j + hh
                                    for cb in range((w + 127) // 128):
                                        rows = min(128, w - 128 * cb)
                                        blk = pB.tile([128, 64], f32, tag="blk",
                                                      name="blk", bufs=3)
                                        nc.sync.dma_start(
                                            blk[0:rows, :],
                                            src_d[h, 512 * c5 + 128 * cb:
                                                  512 * c5 + 128 * cb + rows, :])
                                        tpk = psB.tile([64, 128], f32,
                                                       space="PSUM", tag="tpk",
                                                       name="tpk", bufs=3)
                                        nc.tensor.transpose(
                                            tpk[0:64, 0:rows],
                                            blk[0:rows, :], ident[0:rows, 0:rows])
                                        nc.any.tensor_copy(
                                            kT5[64 * hh:64 * hh + 64,
                                                128 * cb:128 * cb + rows],
                                            tpk[0:64, 0:rows])
                                scps = psB.tile([2, 512], f32, space="PSUM",
                                                tag="scps", name="scps", bufs=2)
                                nc.tensor.matmul(
                                    scps[0:2, 0:w],
                                    q1blk[:, 2 * j:2 * j + 2],
                                    kT5[:, 0:w],
                                    start=True, stop=True)
                                nc.any.tensor_copy(
                                    sct[32 * j:32 * j + 2,
                                        512 * c5:512 * c5 + w],
                                    scps[0:2, 0:w])
                    for h in range(HL):
                        j, hh = h // 2, h % 2
                        nc.sync.dma_start(
                            cbb[8 * h:8 * h + 8, :],
                            sctB[32 * j + hh:32 * j + hh + 1, :]
                            .rearrange("one (c x) -> one c x", c=8))
                        nc.sync.dma_start(
                            cbp[8 * h:8 * h + 8, :],
                            sctP[32 * j + hh:32 * j + hh + 1, :]
                            .rearrange("one (c x) -> one c x", c=8))

                # ===== phase C: qkv GEMM =====
                with tc.tile_pool(name="psC", bufs=3, space="PSUM") as psC:
                    for fc in range(6):
                        dst = qT if fc < 3 else kTc
                        pair = fc % 3
                        for t, n0 in enumerate((0, 512, 1024, 1536)):
                            nn = QT[t]
                            g = psC.tile([128, 512], f32, space="PSUM",
                                         tag="gqk", name="gqk")
                            for cc in range(6):
                                nc.tensor.matmul(
                                    g[:, 0:nn],
                                    wqr[:, cc, 128 * fc:128 * (fc + 1)],
                                    xTr[:, cc, n0:n0 + nn],
                                    start=(cc == 0), stop=(cc == 5))
                            nc.any.tensor_copy(dst[:, pair, n0:n0 + nn],
                                               g[:, 0:nn])
                    nc.vector.memset(v_all[:].bitcast(f32), 0.0)
                    for c in range(17):
                        rows = 128 if c not in (3, 16) else (127 if c == 3 else 32)
                        nc.vector.memset(
                            v_all[0:rows, :, 64:65]
                            .rearrange("p (c6 h) e -> p c6 h e", c6=17)[:, c, :, :]
                            .bitcast(f32),
                            1.0)
                    for c in range(NCH):
                        rows = 128 if c < 12 else 32
                        gv = psC.tile([128, 384], f32, space="PSUM",
                                      tag="gv", name="gv")
                        for cc in range(6):
                            nc.tensor.matmul(
                                gv[0:rows, :],
                                xTr[:, cc, 128 * c:128 * c + rows],
                                wqr[:, cc, 768:1152],
                                start=(cc == 0), stop=(cc == 5))
                        nc.any.tensor_copy(
                            v_all[0:rows, :, 0:64]
                            .rearrange("p (c6 h) e -> p c6 h e", c6=17)[:, 4 + c, :, :],
                            gv[0:rows, :].rearrange("p (h e) -> p h e", h=HL))
            # bigA closed (xTr, wqr freed)

            if DBG:
                nc.sync.dma_start(dbg_q1[:], q1[:])
                nc.sync.dma_start(dbg_sctB[:], sctB[:])
                nc.sync.dma_start(dbg_qT[:], qT[:, :, 0:512].bitcast(f32))
            # ===== phase D: bisection =====
            with tc.tile_pool(name="pD", bufs=1) as pD, \
                 tc.tile_pool(name="psD", bufs=2, space="PSUM") as psD:
                nc.vector.memset(lo[:], -100.0)
                nc.vector.memset(hi[:], 100.0)
                nc.vector.memset(mid[:], 0.0)
                scrb = pD.tile([48, 256], f32)
                scrp = pD.tile([48, 196], f32)
                cnt48b = pD.tile([48, 1], f32)
                cnt48p = pD.tile([48, 1], f32)
                ge = pD.tile([12, 1], i32)
                geinv = pD.tile([12, 1], i32)
                for it in range(NITER):
                    biasb = psD.tile([48, 1], f32, space="PSUM",
                                     tag="biasb", name="biasb", bufs=1)
                    biasp = psD.tile([48, 1], f32, space="PSUM",
                                     tag="biasp", name="biasp", bufs=1)
                    nc.tensor.matmul(biasb[:], B12b[:], mid[:], start=True, stop=True)
                    nc.tensor.matmul(biasp[:], B12p[:], mid[:], start=True, stop=True)
                    bb = pD.tile([48, 1], f32, tag="bb", name="bb", bufs=2)
                    bp = pD.tile([48, 1], f32, tag="bp", name="bp", bufs=2)
                    nc.vector.tensor_copy(bb[:], biasb[:])
                    nc.vector.tensor_copy(bp[:], biasp[:])
                    nc.vector.tensor_scalar(scrb[:], cbb[:], bb[:], 0.0, Alu.is_ge,
                                            Alu.add, accum_out=cnt48b[:])
                    nc.vector.tensor_scalar(scrp[:], cbp[:], bp[:], 0.0, Alu.is_ge,
                                            Alu.add, accum_out=cnt48p[:])
                    cnt12 = psD.tile([12, 1], f32, space="PSUM",
                                     tag="cnt12", name="cnt12", bufs=1)
                    nc.tensor.matmul(cnt12[:], B48b[:], cnt48b[:],
                                     start=True, stop=False)
                    nc.tensor.matmul(cnt12[:], B48p[:], cnt48p[:],
                                     start=False, stop=True)
                    nc.vector.tensor_tensor(ge[:], cnt12[:], kvec[:], op=Alu.is_ge)
                    nc.vector.tensor_tensor(geinv[:], cnt12[:], kvec[:], op=Alu.is_lt)
                    nc.vector.copy_predicated(lo[:], ge[:], mid[:])
                    nc.vector.copy_predicated(hi[:], geinv[:], mid[:])
                    nc.vector.tensor_tensor(mid[:], lo[:], hi[:], op=Alu.add)
                    nc.vector.tensor_scalar_mul(mid[:], mid[:], 0.5)
                tpb_ps = psD.tile([128, 1], f32, space="PSUM", tag="tpbp",
                                  name="tpbp", bufs=1)
                tpp_ps = psD.tile([128, 1], f32, space="PSUM", tag="tppp",
                                  name="tppp", bufs=1)
                nc.tensor.matmul(tpb_ps[:], BmapB[:], lo[:], start=True, stop=True)
                nc.tensor.matmul(tpp_ps[:], BmapP[:], lo[:], start=True, stop=True)
                nc.vector.tensor_copy(tpb[:], tpb_ps[:])
                nc.vector.tensor_copy(tpp[:], tpp_ps[:])

            if DBG:
                nc.sync.dma_start(dbg_lo[:], lo[:])
            # ===== phase E: mask/scan/dest + int16 wrap =====
            w16b_dram = dram.tile([HL, 16, NB // 16], i16)
            w16p_dram = dram.tile([HL, 16, NP // 16], i16)
            with tc.tile_pool(name="pE", bufs=1) as pE:
                maskb = pE.tile([66, NB], i32)
                maskp = pE.tile([66, NP], i32)
                nc.vector.tensor_scalar(maskb[:], sctB[:], tpb[0:66, :], None,
                                        Alu.is_ge)
                nc.vector.tensor_scalar(maskp[:], sctP[:], tpp[0:66, :], None,
                                        Alu.is_ge)
                csb = pE.tile([66, NB], f32)
                csp = pE.tile([66, NP], f32)
                nc.vector.tensor_tensor_scan(csb[:], maskb[:], maskb[:], roffB[:],
                                             op0=Alu.add, op1=Alu.bypass)
                nc.vector.tensor_tensor_scan(csp[:], maskp[:], maskp[:], roffP[:],
                                             op0=Alu.add, op1=Alu.bypass)
                db = pE.tile([66, NB], f32)
                dp = pE.tile([66, NP], f32)
                nc.vector.memset(db[:], float(TRASH))
                nc.vector.memset(dp[:], float(TRASH))
                nc.vector.copy_predicated(db[:], maskb[:], csb[:])
                nc.vector.copy_predicated(dp[:], maskp[:], csp[:])
                dfb_dram = dram.tile([HL, NB], f32)
                dfp_dram = dram.tile([HL, NP], f32)
                for h in range(HL):
                    p = 32 * (h // 2) + h % 2
                    nc.sync.dma_start(dfb_dram[h:h + 1, :], db[p:p + 1, :])
                    nc.sync.dma_start(dfp_dram[h:h + 1, :], dp[p:p + 1, :])
                for h in range(HL):
                    wfb = pE.tile([16, NB // 16], f32, tag="wfb", name="wfb", bufs=2)
                    wfp = pE.tile([16, NP // 16], f32, tag="wfp", name="wfp", bufs=2)
                    nc.sync.dma_start(
                        wfb[:], dfb_dram[h, :].rearrange("(w q) -> q w", q=16))
                    nc.sync.dma_start(
                        wfp[:], dfp_dram[h, :].rearrange("(w q) -> q w", q=16))
                    wib = pE.tile([16, NB // 16], i16, tag="wib", name="wib", bufs=2)
                    wip = pE.tile([16, NP // 16], i16, tag="wip", name="wip", bufs=2)
                    nc.vector.tensor_copy(wib[:], wfb[:])
                    nc.vector.tensor_copy(wip[:], wfp[:])
                    nc.sync.dma_start(w16b_dram[h], wib[:])
                    nc.sync.dma_start(w16p_dram[h], wip[:])
        # bigC closed

        # ===== phase F: kv scatter =====
        sel_dram = dram.tile([TRASH + 2, 128], f32)
        with tc.tile_pool(name="pF", bufs=1) as pF:
            zt = pF.tile([128, 512], f32)
            nc.vector.memset(zt[:], 0.0)
            for h in range(HL):
                base = h * 512
                nc.sync.dma_start(sel_dram[base:base + 128, :], zt[:, 0:128])
                nc.sync.dma_start(sel_dram[base + 128:base + 153, :],
                                  zt[0:25, 0:128])
                base = 3072 + h * 512
                nc.sync.dma_start(sel_dram[base:base + 128, :], zt[:, 0:128])
                nc.sync.dma_start(sel_dram[base + 128:base + 256, :],
                                  zt[:, 128:256])
                nc.sync.dma_start(sel_dram[base + 256:base + 358, :],
                                  zt[0:102, 0:128])
            for h in range(HL):
                kvb = pF.tile([128, 16, 128], f32, tag="kvb", name="kvb", bufs=2)
                nc.sync.dma_start(kvb[:, :, 0:64],
                                  bank_k_d[h].rearrange("(c p) e -> p c e", p=128))
                nc.sync.dma_start(kvb[:, :, 64:128],
                                  bank_v_d[h].rearrange("(c p) e -> p c e", p=128))
                wb = pF.tile([128, NB // 16], i16, tag="wb", name="wb", bufs=2)
                for r in range(8):
                    nc.sync.dma_start(wb[16 * r:16 * (r + 1), :], w16b_dram[h])
                nc.gpsimd.dma_scatter_add(
                    out_ap=sel_dram[:], in_ap=kvb[:], idxs_ap=wb[:],
                    num_idxs=NB, num_idxs_reg=NB, elem_size=128)
                kvp = pF.tile([128, 13, 128], f32, tag="kvp", name="kvp", bufs=2)
                nc.vector.memset(kvp[:, 12:13, :], 0.0)
                nc.sync.dma_start(
                    kvp[:, 0:12, 0:64],
                    prev_k_d[h, 0:1536, :].rearrange("(c p) e -> p c e", p=128))
                nc.sync.dma_start(
                    kvp[0:32, 12:13, 0:64],
                    prev_k_d[h, 1536:1568, :].rearrange("(c p) e -> p c e", p=32))
                nc.sync.dma_start(
                    kvp[:, 0:12, 64:128],
                    prev_v_d[h, 0:1536, :].rearrange("(c p) e -> p c e", p=128))
                nc.sync.dma_start(
                    kvp[0:32, 12:13, 64:128],
                    prev_v_d[h, 1536:1568, :].rearrange("(c p) e -> p c e", p=32))
                wp = pF.tile([128, NP // 16], i16, tag="wp", name="wp", bufs=2)
                for r in range(8):
                    nc.sync.dma_start(wp[16 * r:16 * (r + 1), :], w16p_dram[h])
                nc.gpsimd.dma_scatter_add(
                    out_ap=sel_dram[:], in_ap=kvp[:], idxs_ap=wp[:],
                    num_idxs=NP, num_idxs_reg=NP, elem_size=128)

        if DBG:
            nc.sync.dma_start(dbg_sel[:], sel_dram[0:6144, :])
        # ===== phase G: reload selected kv =====
        with tc.tile_pool(name="pG", bufs=2) as pG, \
             tc.tile_pool(name="psG", bufs=2, space="PSUM") as psG:
            for h in range(HL):
                bb_ = h * 512
                pb_ = 3072 + h * 512
                sk = pG.tile([128, 4, 64], f32, tag="sk", name="sk")
                sv = pG.tile([128, 4, 64], f32, tag="sv", name="sv")
                for (t, half) in ((sk, 0), (sv, 64)):
                    e0, e1 = half, half + 64
                    nc.sync.dma_start(t[:, 0, :], sel_dram[bb_:bb_ + 128, e0:e1])
                    nc.sync.dma_start(t[0:25, 1, :],
                                      sel_dram[bb_ + 128:bb_ + 153, e0:e1])
                    nc.sync.dma_start(t[25:128, 1, :],
                                      sel_dram[pb_:pb_ + 103, e0:e1])
                    nc.sync.dma_start(t[:, 2, :],
                                      sel_dram[pb_ + 103:pb_ + 231, e0:e1])
                    nc.sync.dma_start(t[0:127, 3, :],
                                      sel_dram[pb_ + 231:pb_ + 358, e0:e1])
                kps = psG.tile([64, 512], f32, space="PSUM", tag="kps", name="kps")
                for c in range(4):
                    rows = 128 if c < 3 else 127
                    nc.tensor.transpose(kps[0:64, 128 * c:128 * c + rows],
                                        sk[0:rows, c, :], ident[0:rows, 0:rows])
                nc.any.tensor_copy(
                    kTs[64 * (h % 2):64 * (h % 2) + 64, h // 2, 0:511],
                    kps[0:64, 0:511])
                for c in range(4):
                    rows = 128 if c < 3 else 127
                    nc.any.tensor_copy(
                        v_all[0:rows, :, 0:64]
                        .rearrange("p (c6 hh) e -> p c6 hh e", c6=17)[:, c, h, :],
                        sv[0:rows, c, :])

        # ===== phase H: attention =====
        with tc.tile_pool(name="bigD", bufs=1) as bigD:
            aT = bigD.tile([128, 3, N], f32r)
            with tc.tile_pool(name="pH", bufs=1) as pH, \
                 tc.tile_pool(name="psH", bufs=1, space="PSUM") as psH:
                for h in range(HL):
                    hh = 64 * (h % 2)
                    pr = h // 2
                    for t, n0 in enumerate((0, 512, 1024, 1536)):
                        nn = QT[t]
                        ot = psH.tile([65, 512], f32, space="PSUM",
                                      tag="ot", name="ot", bufs=1)
                        for g in range(6):
                            cs_ = list(range(3 * g, min(3 * g + 3, 17)))
                            sc_ = psH.tile([128, 1536], f32, space="PSUM",
                                           tag="sc", name="sc", bufs=2)
                            for gi, c in enumerate(cs_):
                                mm = MC[c]
                                if c < 4:
                                    lhs = kTs[hh:hh + 64, pr, 128 * c:128 * c + mm]
                                else:
                                    lhs = kTc[hh:hh + 64, pr,
                                              128 * (c - 4):128 * (c - 4) + mm]
                                nc.tensor.matmul(
                                    sc_[0:mm, 512 * gi:512 * gi + nn],
                                    lhs, qT[hh:hh + 64, pr, n0:n0 + nn],
                                    start=True, stop=True)
                            pbt = pH.tile([128, 1536], f32r, tag="pbt",
                                          name="pbt", bufs=3)
                            for gi, c in enumerate(cs_):
                                mm = MC[c]
                                nc.scalar.activation(
                                    pbt[0:mm, 512 * gi:512 * gi + nn],
                                    sc_[0:mm, 512 * gi:512 * gi + nn],
                                    Act.Exp, scale=0.125)
                            for gi, c in enumerate(cs_):
                                mm = MC[c]
                                nc.tensor.matmul(
                                    ot[:, 0:nn],
                                    v_all[0:mm, :, :]
                                    .rearrange("p (c6 h2) e -> p c6 h2 e",
                                               c6=17)[:, c, h, :],
                                    pbt[0:mm, 512 * gi:512 * gi + nn],
                                    start=(c == 0), stop=(c == 16))
                        rcp = pH.tile([1, 512], f32r, tag="rcp", name="rcp", bufs=2)
                        with nc.allow_low_precision(reason="f32r recip for PE"):
                            nc.vector.reciprocal(rcp[0:1, 0:nn], ot[64:65, 0:nn])
                        rps = psH.tile([64, 512], f32, space="PSUM",
                                       tag="rps", name="rps", bufs=1)
                        nc.tensor.matmul(rps[:, 0:nn], ones1[0:1, 0:64],
                                         rcp[0:1, 0:nn], start=True, stop=True)
                        rsb = pH.tile([64, 512], f32, tag="rsb", name="rsb", bufs=2)
                        nc.any.tensor_copy(rsb[:, 0:nn], rps[:, 0:nn])
                        nc.vector.tensor_tensor(aT[hh:hh + 64, pr, n0:n0 + nn],
                                                ot[0:64, 0:nn], rsb[:, 0:nn],
                                                op=Alu.mult)

            if DBG:
                nc.sync.dma_start(dbg_aT[:], aT[:, :, 0:512].bitcast(f32))
            # ===== phase I+J: partial projection + pair ReduceScatter =====
            yloc = dram.tile([N, C], f32)
            with tc.tile_pool(name="pJ", bufs=1) as pJ, \
                 tc.tile_pool(name="psJ", bufs=1, space="PSUM") as psJ:
                wpf = pJ.tile([128, 3, C], f32)
                nc.sync.dma_start(
                    wpf[:], wprojT_d[:].rearrange("(cc p) f -> p cc f", p=128))
                wpr = pJ.tile([128, 3, C], f32r)
                for cc in range(3):
                    nc.vector.tensor_copy(wpr[:, cc, :], wpf[:, cc, :])
                bpf = pJ.tile([1, C], f32)
                nc.sync.dma_start(bpf[:], bproj_d[:])
                bps = psJ.tile([128, C], f32, space="PSUM", tag="bps",
                               name="bps", bufs=1)
                for c0, c1 in ((0, 512), (512, 768)):
                    nc.tensor.matmul(bps[:, c0:c1],
                                     ones1[0:1, :].bitcast(f32),
                                     bpf[:, c0:c1],
                                     start=True, stop=True)
                bias_sb = pJ.tile([128, C], f32)
                nc.any.tensor_copy(bias_sb[:], bps[:])
                for t in range(NCH):
                    rows = 128 if t < 12 else 32
                    yps = psJ.tile([128, C], f32, space="PSUM", tag="yps",
                                   name="yps", bufs=2)
                    for c0, c1 in ((0, 512), (512, 768)):
                        for cc in range(3):
                            nc.tensor.matmul(
                                yps[0:rows, c0:c1],
                                aT[:, cc, 128 * t:128 * t + rows],
                                wpr[:, cc, c0:c1],
                                start=(cc == 0), stop=(cc == 2))
                    ysb = pJ.tile([128, C], f32, tag="ysb", name="ysb", bufs=2)
                    nc.vector.tensor_tensor(ysb[0:rows, :], yps[0:rows, :],
                                            bias_sb[0:rows, :], op=Alu.add)
                    nc.sync.dma_start(yloc[128 * t:128 * t + rows, :],
                                      ysb[0:rows, :])
        # bigD closed

        if DBG:
            with tc.tile_pool(name="pDbgY", bufs=2) as pDY:
                for t in range(NCH):
                    rows = 128 if t < 12 else 32
                    yt2 = pDY.tile([128, C], f32, tag="yt2", name="yt2")
                    nc.sync.dma_start(yt2[0:rows, :], yloc[128 * t:128 * t + rows, :])
                    nc.sync.dma_start(dbg_yloc[128 * t:128 * t + rows, :], yt2[0:rows, :])
        yhalf = dram.tile([NH, C], f32)
        nc.gpsimd.collective_compute(
            "ReduceScatter", Alu.add,
            ins=[yloc[:].opt()],
            outs=[yhalf[:].opt()],
            replica_groups=[[0, 1], [2, 3], [4, 5], [6, 7]],
        )
        with tc.tile_pool(name="pO", bufs=2) as pO:
            for t in range(7):
                rows = 128 if t < 6 else 16
                yt = pO.tile([128, C], f32, tag="yt", name="yt")
                nc.sync.dma_start(yt[0:rows, :], yhalf[128 * t:128 * t + rows, :])
                nc.sync.dma_start(out_d[128 * t:128 * t + rows, :], yt[0:rows, :])
                if DBG:
                    nc.sync.dma_start(dbg_yhalf[128 * t:128 * t + rows, :],
                                      yt[0:rows, :])

    nc.finalize()
    return nc


def _consts():
    B48b = np.zeros((48, 12), np.float32)
    B48p = np.zeros((48, 12), np.float32)
    B12b = np.zeros((12, 48), np.float32)
    B12p = np.zeros((12, 48), np.float32)
    for s in range(6):
        B48b[8 * s:8 * s + 8, s] = 1
        B48p[8 * s:8 * s + 8, 6 + s] = 1
        B12b[s, 8 * s:8 * s + 8] = 1
        B12p[6 + s, 8 * s:8 * s + 8] = 1
    kvec = np.full((12, 1), KP, np.float32)
    kvec[0:6] = KB
    BmapB = np.zeros((12, 128), np.float32)
    BmapP = np.zeros((12, 128), np.float32)
    roffB = np.zeros((66, 1), np.float32)
    roffP = np.zeros((66, 1), np.float32)
    for h in range(6):
        p = 32 * (h // 2) + h % 2
        BmapB[h, p] = 1
        BmapP[6 + h, p] = 1
        roffB[p, 0] = h * 512 - 1
        roffP[p, 0] = 3072 + h * 512 - 1
    ident = np.eye(128, dtype=np.float32)
    return {"B48b": B48b, "B48p": B48p, "B12b": B12b, "B12p": B12p,
            "kvec": kvec, "BmapB": BmapB, "BmapP": BmapP,
            "roffB": roffB, "roffP": roffP, "ident": ident}


def _get_nc():
    if "nc" not in _cache:
        _cache["nc"] = _build()
    return _cache["nc"]


def make_in_maps(x, bank_k, bank_v, prev_k, prev_v, w_qkv, w_proj, b_proj):
    x = np.asarray(x, np.float32)
    bank_k = np.asarray(bank_k, np.float32)
    bank_v = np.asarray(bank_v, np.float32)
    prev_k = np.asarray(prev_k, np.float32)
    prev_v = np.asarray(prev_v, np.float32)
    w_qkv = np.asarray(w_qkv, np.float32)
    w_proj = np.asarray(w_proj, np.float32)
    b_proj = np.asarray(b_proj, np.float32)
    consts = _consts()
    wprojT_full = np.ascontiguousarray(w_proj.T)
    in_maps = []
    for c in range(8):
        b, hg = c // 2, c % 2
        rows = np.concatenate([
            w_qkv[hg * 384:(hg + 1) * 384],
            w_qkv[C + hg * 384:C + (hg + 1) * 384],
            w_qkv[2 * C + hg * 384:2 * C + (hg + 1) * 384]], axis=0)
        m = {
            "x": np.ascontiguousarray(x[b]),
            "wqkvT": np.ascontiguousarray(rows.T),
            "wprojT": np.ascontiguousarray(wprojT_full[hg * 384:(hg + 1) * 384]),
            "bproj": (b_proj.reshape(1, C) if hg == 0
                      else np.zeros((1, C), np.float32)),
            "bank_k": np.ascontiguousarray(bank_k[b, 6 * hg:6 * hg + 6]),
            "bank_v": np.ascontiguousarray(bank_v[b, 6 * hg:6 * hg + 6]),
            "prev_k": np.ascontiguousarray(prev_k[b, 6 * hg:6 * hg + 6]),
            "prev_v": np.ascontiguousarray(prev_v[b, 6 * hg:6 * hg + 6]),
        }
        m.update(consts)
        in_maps.append(m)
    return in_maps


def kernel(x, bank_k, bank_v, prev_k, prev_v, w_qkv, w_proj, b_proj,
           _trace=False):
    from concourse.bass_utils import run_bass_kernel_spmd
    nc = _get_nc()
    in_maps = make_in_maps(x, bank_k, bank_v, prev_k, prev_v,
                           w_qkv, w_proj, b_proj)
    res = run_bass_kernel_spmd(nc, in_maps, core_ids=list(range(8)),
                               trace=_trace)
    out = np.zeros((B, N, C), np.float32)
    for c in range(8):
        b, hg = c // 2, c % 2
        out[b, hg * NH:(hg + 1) * NH, :] = res.results[c]["out"]
    if _trace:
        return out, res
    return out
